# revision 1
# baseline (speedup 1.0000x reference)
"""Multi-head attention Trainium2 kernel (8 NeuronCores).

Sharding: core c handles batch b=c//4 and head group g=c%4 (4 of 16 heads).
Formulation is fully "transposed" so no on-device transposes are needed:
  qT/kT [dq, s] via lhsT=W-pair, rhs=X^T;  v [s, dk] via lhsT=X^T-chunk, rhs=Wv
  scoresT[s_k, s_q] via lhsT=kT-chunk, rhs=qT   (softmax axis = partition dim)
  exp fused on ScalarE (scale=1/sqrt(dq)); rowsum via a ones-column in the
  attn@v matmul; oT[dk, s_q] is exactly the lhsT the output projection wants.
An AllToAll inside each 4-core group reshards from (4 local heads, all s)
to (all 16 heads, s-quarter); each core then computes its final [512, 1024]
output slice and the host concatenates.
"""

import sys

if "/opt/trn_rl_repo" not in sys.path:
    sys.path.insert(0, "/opt/trn_rl_repo")

import numpy as np

import concourse.bass as bass  # noqa: F401  (bass types referenced via tile/bacc)
import concourse.bacc as bacc
import concourse.bass_utils as bass_utils
import concourse.mybir as mybir
import concourse.tile as tile

B, S, DIN = 2, 2048, 1024
H, DK = 16, 64
NCORES = 8
HL = 4  # heads per core
SQ = S // 4  # output rows per core

F32 = mybir.dt.float32
BF16 = mybir.dt.bfloat16

DC = DIN // 128  # 8 din chunks
SKC = S // 128  # 16 s_k chunks
VW = 2 * DK  # 128: 64 v columns + 64 ones columns (rowsum broadcast via PE)


def build(dbg=False):
    nc = bacc.Bacc("TRN2", target_bir_lowering=False, debug=False, num_devices=NCORES)

    xqt = nc.dram_tensor("xqt", [DIN, S], F32, kind="ExternalInput")
    xkt = nc.dram_tensor("xkt", [DIN, S], F32, kind="ExternalInput")
    xvt = nc.dram_tensor("xvt", [DIN, S], F32, kind="ExternalInput")
    wq = nc.dram_tensor("wq", [DIN, HL * DK], F32, kind="ExternalInput")
    wk = nc.dram_tensor("wk", [DIN, HL * DK], F32, kind="ExternalInput")
    wv = nc.dram_tensor("wv", [DIN, HL * DK], F32, kind="ExternalInput")
    # Wo with zero rows for the other batch's AllToAll blocks: [2*H*DK, DIN]
    wo = nc.dram_tensor("wo", [2 * H * DK, DIN], F32, kind="ExternalInput")
    bqp = nc.dram_tensor("bqp", [128, 2], F32, kind="ExternalInput")
    bkp = nc.dram_tensor("bkp", [128, 2], F32, kind="ExternalInput")
    bvr = nc.dram_tensor("bvr", [128, HL * DK], F32, kind="ExternalInput")
    bor = nc.dram_tensor("bor", [128, DIN], F32, kind="ExternalInput")
    out = nc.dram_tensor("out", [SQ, DIN], F32, kind="ExternalOutput")
    if dbg:
        d_qt = nc.dram_tensor("d_qt", [128, S], BF16, kind="ExternalOutput")
        d_kt = nc.dram_tensor("d_kt", [128, S], BF16, kind="ExternalOutput")
        d_v = nc.dram_tensor("d_v", [128, HL * VW], BF16, kind="ExternalOutput")
        d_ccin = nc.dram_tensor("d_ccin", [8 * HL * DK, SQ], BF16, kind="ExternalOutput")
        d_ccout = nc.dram_tensor("d_ccout", [8 * HL * DK, SQ], BF16, kind="ExternalOutput")
        d_rsum = nc.dram_tensor("d_rsum", [64, 1024], F32, kind="ExternalOutput")
        d_rlo = nc.dram_tensor("d_rlo", [64, 1024], F32, kind="ExternalOutput")
        d_et = nc.dram_tensor("d_et", [128, 1024], BF16, kind="ExternalOutput")

    with tile.TileContext(nc) as tc:
        with (
            tc.tile_pool(name="pers", bufs=1) as pers,
            tc.tile_pool(name="work", bufs=3) as work,
            tc.tile_pool(name="wrk2", bufs=2) as wrk2,
            tc.tile_pool(name="psmm", bufs=2, space="PSUM") as psmm,
            tc.tile_pool(name="psacc", bufs=3, space="PSUM") as psacc,
            tc.tile_pool(name="pspj", bufs=1, space="PSUM") as pspj,
            tc.tile_pool(name="dram", bufs=1, space="DRAM") as dram,
        ):
            # ---- load weights/biases (cast fp32 -> bf16 where used by PE) ----
            wq_sb = pers.tile([128, DC, HL * DK], BF16)
            wk_sb = pers.tile([128, DC, HL * DK], BF16)
            wv_sb = pers.tile([128, DC, HL * DK], BF16)
            nc.gpsimd.dma_start(wq_sb[:], wq.rearrange("(c p) d -> p c d", p=128))
            nc.gpsimd.dma_start(wk_sb[:], wk.rearrange("(c p) d -> p c d", p=128))
            nc.gpsimd.dma_start(wv_sb[:], wv.rearrange("(c p) d -> p c d", p=128))
            bq_sb = pers.tile([128, 2], F32)
            bk_sb = pers.tile([128, 2], F32)
            bv_sb = pers.tile([128, HL * DK], F32)
            bo_sb = pers.tile([128, DIN], F32)
            nc.sync.dma_start(bq_sb[:], bqp[:])
            nc.sync.dma_start(bk_sb[:], bkp[:])
            nc.sync.dma_start(bv_sb[:], bvr[:])
            nc.sync.dma_start(bo_sb[:], bor[:])

            # ---- load X^T (cast to bf16), streamed per s-block so the
            # projections and attention can start before loads finish ----
            # "big" tag slots get recycled for wo_sb / ol_sb after projections
            xq_sb = pers.tile([128, DC, S], BF16, tag="big", bufs=3, name="xq_sb")
            xk_sb = pers.tile([128, DC, S], BF16, tag="big", bufs=3, name="xk_sb")
            xv_sb = pers.tile([128, DC, S], BF16, tag="big", bufs=3, name="xv_sb")
            for sblk in range(4):
                ssl = slice(512 * sblk, 512 * (sblk + 1))
                for xsb, xdram in ((xq_sb, xqt), (xk_sb, xkt), (xv_sb, xvt)):
                    nc.gpsimd.dma_start(
                        xsb[:, :, ssl],
                        xdram[:, ssl].rearrange("(c p) s -> p c s", p=128),
                    )

            # ---- projections ----
            # qT/kT: [128 = pair of heads (2*64), S] per head-pair
            qt_sb = [pers.tile([128, S], BF16, name=f"qt{p}") for p in range(2)]
            kt_sb = [pers.tile([128, S], BF16, name=f"kt{p}") for p in range(2)]

            def emit_qk(p, sblks=range(4)):
                for xsb, wsb, bsb, dst in (
                    (xq_sb, wq_sb, bq_sb, qt_sb),
                    (xk_sb, wk_sb, bk_sb, kt_sb),
                ):
                    for sb in sblks:
                        ps = pspj.tile([128, 512], F32, tag="pj", name="psqk")
                        for c in range(DC):
                            nc.tensor.matmul(
                                ps[:],
                                wsb[:, c, 128 * p : 128 * (p + 1)],
                                xsb[:, c, 512 * sb : 512 * (sb + 1)],
                                start=(c == 0),
                                stop=(c == DC - 1),
                            )
                        nc.vector.tensor_scalar_add(
                            dst[p][:, 512 * sb : 512 * (sb + 1)], ps[:], bsb[:, p : p + 1]
                        )

            # v: [s (partitions, 16 chunks), 4 heads x (64 v cols + 64 ones cols)]
            v_sb = pers.tile([128, SKC, HL * VW], BF16)

            def emit_v_ones():
                for h in range(HL):
                    nc.vector.memset(v_sb[:, :, h * VW + DK : (h + 1) * VW], 1.0)

            def emit_v(scs):
                for sc in scs:
                    psv = pspj.tile([128, HL * DK], F32, tag="pj", name="psv")
                    for c in range(DC):
                        nc.tensor.matmul(
                            psv[:],
                            xv_sb[:, c, 128 * sc : 128 * (sc + 1)],
                            wv_sb[:, c, :],
                            start=(c == 0),
                            stop=(c == DC - 1),
                        )
                    for h in range(HL):
                        nc.vector.tensor_add(
                            v_sb[:, sc, h * VW : h * VW + DK],
                            psv[:, h * DK : (h + 1) * DK],
                            bv_sb[:, h * DK : (h + 1) * DK],
                        )

            # ---- attention + collective input staging ----
            # per head-pair: 8 shards of 128 rows (2 heads x 64); shard j
            # carries quarter j%4 (written twice, once per batch's range)
            cc_in = [
                dram.tile([8 * 2 * DK, SQ], BF16, name=f"cc_in{p}") for p in range(2)
            ]
            cc_out = [
                dram.tile([8 * 2 * DK, SQ], BF16, name=f"cc_out{p}") for p in range(2)
            ]

            def emit_a2a(p):
                nc.gpsimd.collective_compute(
                    "AllToAll",
                    mybir.AluOpType.bypass,
                    replica_groups=[[0, 1, 2, 3, 4, 5, 6, 7]],
                    ins=[cc_in[p].opt()],
                    outs=[cc_out[p].opt()],
                )

            def emit_attention(p):
                for sqb in range(4):  # s_q quarters of 512
                    qsl = slice(512 * sqb, 512 * (sqb + 1))
                    # po[ch]: [0:64]=oT, [64:128]=rowsum (ones block)
                    po = [
                        psacc.tile([128, 512], F32, tag="acc", name=f"po{ch}")
                        for ch in range(2)
                    ]
                    for skc in range(SKC):
                        # one tile for the head pair: A scores in [:, 0:512]
                        # (bank 0), B scores in [:, 512:1024] (bank 1).
                        # Shared slot dependency -> the two row-tiled MMs
                        # issue adjacently and run concurrently.
                        ps2 = psmm.tile([128, 1024], F32, tag="mm", name="ps2")
                        for ch in range(2):
                            cs = slice(64 * ch, 64 * (ch + 1))
                            nc.tensor.matmul(
                                ps2[:, 512 * ch : 512 * (ch + 1)],
                                kt_sb[p][cs, 128 * skc : 128 * (skc + 1)],
                                qt_sb[p][cs, qsl],
                                start=True,
                                stop=True,
                            )
                        et = work.tile([128, 1024], BF16, tag="et", name="et")
                        nc.scalar.activation(
                            et[:],
                            ps2[:],
                            mybir.ActivationFunctionType.Exp,
                            bias=0.0,
                            scale=float(1.0 / np.sqrt(DK)),
                        )
                        if dbg and p == 0 and sqb == 0 and skc == 0:
                            nc.sync.dma_start(d_et[:], et[:])
                        for ch in range(2):
                            h = 2 * p + ch
                            nc.tensor.matmul(
                                po[ch][:],
                                v_sb[:, skc, h * VW : h * VW + VW],
                                et[:, 512 * ch : 512 * (ch + 1)],
                                start=(skc == 0),
                                stop=(skc == SKC - 1),
                            )
                    for ch in range(2):
                        h = 2 * p + ch
                        rcp = wrk2.tile([128, 512], F32, tag="rcp", name="rcp")
                        rlo = wrk2.tile([64, 512], F32, tag="rlo", name="rlo")
                        ot = wrk2.tile([64, 512], BF16, tag="ot", name="ot")
                        nc.vector.reciprocal_approx_fast(out=rcp[:], in_=po[ch][:])
                        # shift rowsum reciprocals down to partitions 0..63
                        nc.sync.dma_start(rlo[:], rcp[64:128, :])
                        if dbg and h == 0 and sqb == 0:
                            rsd = wrk2.tile([128, 512], F32, tag="rsd", name="rsd")
                            nc.vector.tensor_copy(rsd[64:128, :], po[ch][64:128, :])
                            nc.sync.dma_start(d_rsum[:, 0:512], rsd[64:128, :])
                            nc.sync.dma_start(d_rlo[:, 0:512], rlo[:])
                        nc.vector.tensor_mul(ot[:], po[ch][0:DK, :], rlo[:])
                        for shard in (sqb, sqb + 4):
                            base = shard * 2 * DK + ch * DK
                            nc.sync.dma_start(cc_in[p][base : base + DK, :], ot[:])

            emit_v_ones()
            # proj emission follows the s-block streaming order of the loads
            for sblk in range(4):
                emit_qk(0, [sblk])
                emit_v(range(4 * sblk, 4 * sblk + 4))
            emit_attention(0)
            emit_qk(1)  # fills PE gaps during pair-0 attention
            # wo load (slot freed by xq after pair-1 proj); runs during attention
            wo_sb = pers.tile([128, 2 * DC, DIN], BF16, tag="big", bufs=3, name="wo_sb")
            nc.gpsimd.dma_start(wo_sb[:], wo.rearrange("(c p) d -> p c d", p=128))
            emit_a2a(0)  # overlaps pair-1 attention
            ol_sb = pers.tile([128, 2 * DC, SQ], BF16, tag="big", bufs=3, name="ol_sb")
            nc.gpsimd.dma_start(
                ol_sb[:, 0:DC, :], cc_out[0].rearrange("(c p) s -> p c s", p=128)
            )
            emit_attention(1)
            emit_a2a(1)
            nc.gpsimd.dma_start(
                ol_sb[:, DC : 2 * DC, :], cc_out[1].rearrange("(c p) s -> p c s", p=128)
            )


            if dbg:
                nc.sync.dma_start(d_ccin[0 : 8 * 2 * DK, :], cc_in[0][:])
                nc.sync.dma_start(d_ccin[8 * 2 * DK :, :], cc_in[1][:])
                nc.sync.dma_start(d_ccout[0 : 8 * 2 * DK, :], cc_out[0][:])
                nc.sync.dma_start(d_ccout[8 * 2 * DK :, :], cc_out[1][:])

            # ---- output projection for this core's s-quarter ----
            for sb2 in range(SQ // 128):
                os_sb = wrk2.tile([128, DIN], F32, tag="os", name="os")
                for do in range(2):
                    g = 2 * sb2 + do
                    pool = psmm if g % 3 < 2 else pspj
                    pso = pool.tile(
                        [128, 512], F32, tag="mm" if g % 3 < 2 else "pj", name="pso"
                    )
                    for c in range(2 * DC):
                        nc.tensor.matmul(
                            pso[:],
                            ol_sb[:, c, 128 * sb2 : 128 * (sb2 + 1)],
                            wo_sb[:, c, 512 * do : 512 * (do + 1)],
                            start=(c == 0),
                            stop=(c == 2 * DC - 1),
                        )
                    nc.vector.tensor_add(
                        os_sb[:, 512 * do : 512 * (do + 1)],
                        pso[:],
                        bo_sb[:, 512 * do : 512 * (do + 1)],
                    )
                nc.sync.dma_start(out[128 * sb2 : 128 * (sb2 + 1), :], os_sb[:])

    nc.compile()
    return nc


_NC = None


def _get_nc():
    global _NC
    if _NC is None:
        _NC = build()
    return _NC


def _pack_wo(Wo, b):
    """Row order must match the ol_sb contraction layout: chunks 0-7 are
    AllToAll block rows (rank i, pair-0 heads), chunks 8-15 pair-1 heads.
    Rows for ranks of the other batch are zeroed (they carry that batch's
    data in cc_out and must not contribute)."""
    out = np.zeros((2 * H * DK, DIN), np.float32)
    for p in range(2):
        for i in range(8):
            if i // 4 != b:
                continue
            for hh in range(2):
                hg = 4 * (i % 4) + 2 * p + hh
                dst = 1024 * p + 128 * i + 64 * hh
                out[dst : dst + 64, :] = Wo[hg * 64 : (hg + 1) * 64, :]
    return out


def make_in_maps(Q, K, V, Wq, bq, Wk, bk, Wv, bv, Wo, bo):
    Q, K, V = (np.asarray(a, np.float32) for a in (Q, K, V))
    Wq, bq, Wk, bk, Wv, bv = (
        np.asarray(a, np.float32) for a in (Wq, bq, Wk, bk, Wv, bv)
    )
    Wo = np.asarray(Wo, np.float32)
    bo = np.asarray(bo, np.float32)
    in_maps = []
    for c in range(NCORES):
        b, g = divmod(c, 4)
        hs = slice(HL * g, HL * (g + 1))
        # head-pair-stacked per-partition bias vectors [128, 2]
        bq2 = np.ascontiguousarray(bq[hs].reshape(2, 128).T)
        bk2 = np.ascontiguousarray(bk[hs].reshape(2, 128).T)
        in_maps.append(
            {
                "xqt": np.ascontiguousarray(Q[b].T),
                "xkt": np.ascontiguousarray(K[b].T),
                "xvt": np.ascontiguousarray(V[b].T),
                "wq": np.ascontiguousarray(
                    Wq[hs].transpose(1, 0, 2).reshape(DIN, HL * DK)
                ),
                "wk": np.ascontiguousarray(
                    Wk[hs].transpose(1, 0, 2).reshape(DIN, HL * DK)
                ),
                "wv": np.ascontiguousarray(
                    Wv[hs].transpose(1, 0, 2).reshape(DIN, HL * DK)
                ),
                "wo": _pack_wo(Wo, b),
                "bqp": bq2,
                "bkp": bk2,
                "bvr": np.ascontiguousarray(
                    np.broadcast_to(bv[hs].reshape(-1), (128, HL * DK))
                ),
                "bor": np.ascontiguousarray(np.broadcast_to(bo, (128, DIN))),
            }
        )
    return in_maps


def run(nc, in_maps, **kwargs):
    return bass_utils.run_bass_kernel_spmd(
        nc, in_maps, core_ids=list(range(NCORES)), **kwargs
    )


def kernel(Q, K, V, Wq, bq, Wk, bk, Wv, bv, Wo, bo):
    nc = _get_nc()
    in_maps = make_in_maps(Q, K, V, Wq, bq, Wk, bk, Wv, bv, Wo, bo)
    res = run(nc, in_maps)
    full = np.empty((B, S, DIN), np.float32)
    for c in range(NCORES):
        b, g = divmod(c, 4)
        full[b, SQ * g : SQ * (g + 1), :] = res.results[c]["out"]
    return full



# revision 21
# speedup vs baseline: 1.0615x; 1.0615x over previous
"""Multi-head attention Trainium2 kernel (8 NeuronCores).

Sharding: core c handles batch b=c//4 and head group g=c%4 (4 of 16 heads).
Fully "transposed" formulation (no on-device transposes):
  qT/kT [dq, s] via lhsT=W-pair, rhs=X^T;  v [s, dk] via lhsT=X^T-chunk, rhs=Wv
  scoresT[s_k, s_q] via lhsT=kT-chunk, rhs=qT (two 64-row head MMs run
  concurrently on the PE via row tiling); exp fused PSUM->SBUF on ScalarE
  (scale=1/sqrt(dq)), with a fraction of tiles offloaded to the Vector engine
  using a Schraudolph bit-trick fast-exp (x*A+B written as int16, bitcast to
  bf16); rowsum via a ones-column block in the attn@v matmul (free: matmul
  cost depends only on moving columns); oT[dk, s_q] is exactly the lhsT the
  output projection wants.
An AllToAll across all 8 cores reshards from (4 local heads, all s) to
(all 16 heads, s-quarter); Wo is zero-padded for the other batch's shard rows
(4-rank groups are unsupported: the mesh algorithm needs >4 ranks). The pair-0
half of the output projection is emitted before the pair-1 AllToAll completes
so the PE works during the collective. Inputs are cast to bf16 host-side to
halve HBM load traffic.
"""

import sys

if "/opt/trn_rl_repo" not in sys.path:
    sys.path.insert(0, "/opt/trn_rl_repo")

import ml_dtypes
import numpy as np

import concourse.bass as bass  # noqa: F401
import concourse.bacc as bacc
import concourse.bass_utils as bass_utils
import concourse.mybir as mybir
import concourse.tile as tile

B, S, DIN = 2, 2048, 1024
H, DK = 16, 64
NCORES = 8
HL = 4  # heads per core
SQ = S // 4  # output rows per core

F32 = mybir.dt.float32
BF16 = mybir.dt.bfloat16
I16 = mybir.dt.int16

DC = DIN // 128  # 8 din chunks
SKC = S // 128  # 16 s_k chunks
VW = 2 * DK  # 128: 64 v columns + 64 ones columns (rowsum via PE)

SCALE = float(1.0 / np.sqrt(DK))
# Schraudolph fast-exp producing bf16 bit patterns: bits = int16(x*FE_A + FE_B)
FE_A = float(128.0 / np.log(2.0)) * SCALE
FE_B = float(127.0 * 128.0 - 0.5)
# skc chunks whose exp runs on the Vector engine instead of ScalarE
DVE_SKC = (1, 5, 9, 13)


def build(dbg=False):
    nc = bacc.Bacc("TRN2", target_bir_lowering=False, debug=False, num_devices=NCORES)

    xqt = nc.dram_tensor("xqt", [DIN, S], BF16, kind="ExternalInput")
    xkt = nc.dram_tensor("xkt", [DIN, S], BF16, kind="ExternalInput")
    xvt = nc.dram_tensor("xvt", [DIN, S], BF16, kind="ExternalInput")
    wq = nc.dram_tensor("wq", [DIN, HL * DK], BF16, kind="ExternalInput")
    wk = nc.dram_tensor("wk", [DIN, HL * DK], BF16, kind="ExternalInput")
    wv = nc.dram_tensor("wv", [DIN, HL * DK], BF16, kind="ExternalInput")
    # Wo in AllToAll row order with zero rows for the other batch's shards:
    # chunk 8p+i = heads (4*(i%4)+2p, +1) of rank i if i//4==b else zeros
    wo = nc.dram_tensor("wo", [2 * H * DK, DIN], BF16, kind="ExternalInput")
    bqp = nc.dram_tensor("bqp", [128, 2], F32, kind="ExternalInput")
    bkp = nc.dram_tensor("bkp", [128, 2], F32, kind="ExternalInput")
    bvr = nc.dram_tensor("bvr", [128, HL * DK], F32, kind="ExternalInput")
    bor = nc.dram_tensor("bor", [128, DIN], F32, kind="ExternalInput")
    out = nc.dram_tensor("out", [SQ, DIN], F32, kind="ExternalOutput")
    if dbg:
        d_qt = nc.dram_tensor("d_qt", [128, S], BF16, kind="ExternalOutput")
        d_kt = nc.dram_tensor("d_kt", [128, S], BF16, kind="ExternalOutput")
        d_v = nc.dram_tensor("d_v", [128, HL * VW], BF16, kind="ExternalOutput")
        d_eta = nc.dram_tensor("d_eta", [128, 1024], BF16, kind="ExternalOutput")
        d_etd = nc.dram_tensor("d_etd", [128, 1024], BF16, kind="ExternalOutput")
        d_po = nc.dram_tensor("d_po", [128, 1024], F32, kind="ExternalOutput")
        d_ot = nc.dram_tensor("d_ot", [64, 1024], BF16, kind="ExternalOutput")
        d_ol = nc.dram_tensor("d_ol", [128, 2 * DC * SQ], BF16, kind="ExternalOutput")
        d_wo = nc.dram_tensor("d_wo", [128, DIN], BF16, kind="ExternalOutput")

    with tile.TileContext(nc) as tc:
        with (
            tc.tile_pool(name="pers", bufs=1) as pers,
            tc.tile_pool(name="work", bufs=3) as work,
            tc.tile_pool(name="wrk2", bufs=2) as wrk2,
            tc.tile_pool(name="pmm", bufs=2, space="PSUM") as pmm,
            tc.tile_pool(name="pacc", bufs=2, space="PSUM") as pacc,
            tc.tile_pool(name="dram", bufs=1, space="DRAM") as dram,
        ):
            # ---- weights/biases; q/k path on sync queue, v path on gpsimd ----
            wq_sb = pers.tile([128, DC, HL * DK], BF16)
            wk_sb = pers.tile([128, DC, HL * DK], BF16)
            wv_sb = pers.tile([128, DC, HL * DK], BF16)
            bq_sb = pers.tile([128, 2], F32)
            bk_sb = pers.tile([128, 2], F32)
            bv_sb = pers.tile([128, HL * DK], F32)
            bo_sb = pers.tile([128, DIN], F32)
            nc.sync.dma_start(wq_sb[:], wq.rearrange("(c p) d -> p c d", p=128))
            nc.sync.dma_start(wk_sb[:], wk.rearrange("(c p) d -> p c d", p=128))
            nc.sync.dma_start(bq_sb[:], bqp[:])
            nc.sync.dma_start(bk_sb[:], bkp[:])
            nc.gpsimd.dma_start(wv_sb[:], wv.rearrange("(c p) d -> p c d", p=128))
            nc.gpsimd.dma_start(bv_sb[:], bvr[:])
            nc.gpsimd.dma_start(bo_sb[:], bor[:])

            # ---- X^T streamed per 512-col s-block; "big" slots recycled for
            # wo_sb / ol_sb after the projections are done ----
            xq_sb = pers.tile([128, DC, S], BF16, tag="big", bufs=3, name="xq_sb")
            xk_sb = pers.tile([128, DC, S], BF16, tag="big", bufs=3, name="xk_sb")
            xv_sb = pers.tile([128, DC, S], BF16, tag="big", bufs=3, name="xv_sb")
            for sblk in range(4):
                ssl = slice(512 * sblk, 512 * (sblk + 1))
                nc.sync.dma_start(
                    xq_sb[:, :, ssl], xqt[:, ssl].rearrange("(c p) s -> p c s", p=128)
                )
                nc.sync.dma_start(
                    xk_sb[:, :, ssl], xkt[:, ssl].rearrange("(c p) s -> p c s", p=128)
                )
                nc.gpsimd.dma_start(
                    xv_sb[:, :, ssl], xvt[:, ssl].rearrange("(c p) s -> p c s", p=128)
                )

            # ---- projection targets ----
            qt_sb = [pers.tile([128, S], BF16, name=f"qt{p}") for p in range(2)]
            kt_sb = [pers.tile([128, S], BF16, name=f"kt{p}") for p in range(2)]
            v_sb = pers.tile([128, SKC, HL * VW], BF16)

            def emit_qk(p, sblks):
                for xsb, wsb, bsb, dst in (
                    (xq_sb, wq_sb, bq_sb, qt_sb),
                    (xk_sb, wk_sb, bk_sb, kt_sb),
                ):
                    for sb in sblks:
                        ps = pmm.tile([128, 1024], F32, tag="mm", name="psqk")
                        for c in range(DC):
                            nc.tensor.matmul(
                                ps[:, 0:512],
                                wsb[:, c, 128 * p : 128 * (p + 1)],
                                xsb[:, c, 512 * sb : 512 * (sb + 1)],
                                start=(c == 0),
                                stop=(c == DC - 1),
                            )
                        nc.vector.tensor_scalar_add(
                            dst[p][:, 512 * sb : 512 * (sb + 1)],
                            ps[:, 0:512],
                            bsb[:, p : p + 1],
                        )

            def emit_v_ones():
                for hl in range(HL):
                    nc.vector.memset(v_sb[:, :, hl * VW + DK : (hl + 1) * VW], 1.0)

            def emit_v(scs):
                for sc in scs:
                    psv = pmm.tile([128, 1024], F32, tag="mm", name="psv")
                    for c in range(DC):
                        nc.tensor.matmul(
                            psv[:, 0 : HL * DK],
                            xv_sb[:, c, 128 * sc : 128 * (sc + 1)],
                            wv_sb[:, c, :],
                            start=(c == 0),
                            stop=(c == DC - 1),
                        )
                    for hl in range(HL):
                        nc.vector.tensor_add(
                            v_sb[:, sc, hl * VW : hl * VW + DK],
                            psv[:, hl * DK : (hl + 1) * DK],
                            bv_sb[:, hl * DK : (hl + 1) * DK],
                        )

            # ---- attention + collective staging ----
            # per pair: 8 shards of 128 rows (2 heads x 64); shard j carries
            # s_q quarter j%4, written twice (once per batch's rank range)
            cc_in = [dram.tile([8 * VW, SQ], BF16, name=f"cc_in{p}") for p in range(2)]
            cc_out = [
                dram.tile([8 * VW, SQ], BF16, name=f"cc_out{p}") for p in range(2)
            ]

            def emit_a2a(p):
                nc.gpsimd.collective_compute(
                    "AllToAll",
                    mybir.AluOpType.bypass,
                    replica_groups=[[0, 1, 2, 3, 4, 5, 6, 7]],
                    ins=[cc_in[p].opt()],
                    outs=[cc_out[p].opt()],
                )

            def emit_att_block(p, sqb, skcs, po):
                qsl = slice(512 * sqb, 512 * (sqb + 1))
                for skc in skcs:
                    # [0:512]=head 2p scores (bank A), [512:1024]=head 2p+1
                    # (bank B); shared tile -> the two 64-row MMs issue
                    # adjacently and run concurrently on the PE
                    ps2 = pmm.tile([128, 1024], F32, tag="mm", name="ps2")
                    for ch in range(2):
                        cs = slice(64 * ch, 64 * (ch + 1))
                        nc.tensor.matmul(
                            ps2[:, 512 * ch : 512 * (ch + 1)],
                            kt_sb[p][cs, 128 * skc : 128 * (skc + 1)],
                            qt_sb[p][cs, qsl],
                            start=True,
                            stop=True,
                        )
                    et = work.tile([128, 1024], BF16, tag="et", name="et")
                    if skc in DVE_SKC:
                        nc.vector.tensor_scalar(
                            et.bitcast(I16),
                            ps2[:],
                            FE_A,
                            FE_B,
                            mybir.AluOpType.mult,
                            mybir.AluOpType.add,
                        )
                    else:
                        nc.scalar.activation(
                            et[:],
                            ps2[:],
                            mybir.ActivationFunctionType.Exp,
                            bias=0.0,
                            scale=SCALE,
                        )
                    if dbg and p == 0 and sqb == 1 and skc == 0:
                        nc.sync.dma_start(d_eta[:], et[:])
                    if dbg and p == 0 and sqb == 1 and skc == 1:
                        nc.sync.dma_start(d_etd[:], et[:])
                    for ch in range(2):
                        hl = 2 * p + ch
                        nc.tensor.matmul(
                            po[:, 512 * ch : 512 * (ch + 1)],
                            v_sb[:, skc, hl * VW : (hl + 1) * VW],
                            et[:, 512 * ch : 512 * (ch + 1)],
                            start=(skc == 0),
                            stop=(skc == SKC - 1),
                        )

            def emit_att_norm(p, sqb, po):
                # po rows 0:64 = oT, rows 64:128 = rowsum (ones block).
                if dbg and p == 0 and sqb == 1:
                    po_sb = wrk2.tile([128, 1024], F32, tag="rcp", name="po_sb")
                    nc.vector.tensor_copy(po_sb[:], po[:])
                    nc.sync.dma_start(d_po[:], po_sb[:])
                rcp = wrk2.tile([128, 1024], F32, tag="rcp", name="rcp")
                rlo = wrk2.tile([64, 1024], F32, tag="rlo", name="rlo")
                ot = wrk2.tile([64, 1024], BF16, tag="ot", name="ot")
                # full-tile rcp: the custom-DVE op mishandles partition-offset
                # ranges on HW (rows 0:64 are unused garbage reciprocals)
                nc.vector.reciprocal_approx_fast(out=rcp[:], in_=po[:])
                nc.sync.dma_start(rlo[:], rcp[64:128, :])
                nc.vector.tensor_mul(ot[:], po[0:DK, :], rlo[:])
                if dbg and p == 0 and sqb == 1:
                    nc.sync.dma_start(d_ot[:], ot[:])
                for shard in (sqb, sqb + 4):
                    for ch in range(2):
                        base = 128 * shard + 64 * ch
                        nc.gpsimd.dma_start(
                            cc_in[p][base : base + 64, :],
                            ot[:, 512 * ch : 512 * (ch + 1)],
                        )

            emit_v_ones()
            # pair-0: stream projections per s-block and start attention on
            # s_q quarter 0 as soon as each kt/v block lands
            po00 = pacc.tile([128, 1024], F32, tag="acc", name="po")
            for sblk in range(4):
                emit_qk(0, [sblk])
                emit_v(range(4 * sblk, 4 * sblk + 4))
                emit_att_block(0, 0, range(4 * sblk, 4 * sblk + 4), po00)
            emit_att_norm(0, 0, po00)
            for sqb in range(1, 4):
                po = pacc.tile([128, 1024], F32, tag="acc", name="po")
                emit_att_block(0, sqb, range(SKC), po)
                emit_att_norm(0, sqb, po)
            emit_qk(1, range(4))  # fills PE gaps while pair-0 exp drains
            # wo load (slot freed by xq after pair-1 proj); runs during attention
            wo_sb = pers.tile([128, 2 * DC, DIN], BF16, tag="big", bufs=3, name="wo_sb")
            nc.sync.dma_start(wo_sb[:], wo.rearrange("(c p) d -> p c d", p=128))
            emit_a2a(0)  # overlaps pair-1 attention
            ol_sb = pers.tile([128, 2 * DC, SQ], BF16, tag="big", bufs=3, name="ol_sb")
            nc.gpsimd.dma_start(
                ol_sb[:, 0:DC, :], cc_out[0].rearrange("(c p) s -> p c s", p=128)
            )
            for sqb in range(4):
                po = pacc.tile([128, 1024], F32, tag="acc", name="po")
                emit_att_block(1, sqb, range(SKC), po)
                emit_att_norm(1, sqb, po)
            emit_a2a(1)

            # ---- output projection: pair-0 half runs during the pair-1
            # AllToAll; pair-1 half accumulates into the same PSUM after ----
            pso_tiles = []
            for sb2 in range(4):
                pool, tg = (pmm, "mm") if sb2 < 2 else (pacc, "acc")
                pso = pool.tile([128, 1024], F32, tag=tg, name="pso")
                for c in range(DC):
                    for do in range(2):
                        nc.tensor.matmul(
                            pso[:, 512 * do : 512 * (do + 1)],
                            ol_sb[:, c, 128 * sb2 : 128 * (sb2 + 1)],
                            wo_sb[:, c, 512 * do : 512 * (do + 1)],
                            start=(c == 0),
                            stop=False,
                        )
                pso_tiles.append(pso)
            nc.gpsimd.dma_start(
                ol_sb[:, DC : 2 * DC, :],
                cc_out[1].rearrange("(c p) s -> p c s", p=128),
            )
            if dbg:
                nc.sync.dma_start(d_qt[:], qt_sb[0][:])
                nc.sync.dma_start(d_kt[:], kt_sb[0][:])
                nc.sync.dma_start(d_v[:], v_sb[:, 0, :])
                nc.sync.dma_start(d_wo[:], wo_sb[:, 8, :])
                nc.sync.dma_start(
                    d_ol.rearrange("p (c s) -> p c s", c=2 * DC), ol_sb[:]
                )
            for sb2 in range(4):
                pso = pso_tiles[sb2]
                for c in range(DC, 2 * DC):
                    for do in range(2):
                        nc.tensor.matmul(
                            pso[:, 512 * do : 512 * (do + 1)],
                            ol_sb[:, c, 128 * sb2 : 128 * (sb2 + 1)],
                            wo_sb[:, c, 512 * do : 512 * (do + 1)],
                            start=False,
                            stop=(c == 2 * DC - 1),
                        )
                os_sb = wrk2.tile([128, DIN], F32, tag="os", name="os")
                nc.vector.tensor_add(os_sb[:], pso[:], bo_sb[:])
                nc.sync.dma_start(out[128 * sb2 : 128 * (sb2 + 1), :], os_sb[:])

    nc.compile()
    return nc


_NC = None


def _get_nc():
    global _NC
    if _NC is None:
        _NC = build()
    return _NC


def _pack_wo(Wo, b):
    """Row order matches the AllToAll output chunks: chunk 8p+i (128 rows)
    holds heads (4*(i%4)+2p, +1) of rank i. Rows for the other batch's ranks
    are zeroed (they carry that batch's data in cc_out and must not
    contribute)."""
    bf = ml_dtypes.bfloat16
    out = np.zeros((2 * H * DK, DIN), bf)
    for p in range(2):
        for i in range(8):
            if i // 4 != b:
                continue
            for hh in range(2):
                head = 4 * (i % 4) + 2 * p + hh
                dst = 1024 * p + 128 * i + 64 * hh
                out[dst : dst + 64, :] = Wo[head * 64 : (head + 1) * 64, :].astype(bf)
    return out


def make_in_maps(Q, K, V, Wq, bq, Wk, bk, Wv, bv, Wo, bo):
    bf = ml_dtypes.bfloat16
    Q, K, V = (np.asarray(a, np.float32) for a in (Q, K, V))
    Wq, bq, Wk, bk, Wv, bv = (
        np.asarray(a, np.float32) for a in (Wq, bq, Wk, bk, Wv, bv)
    )
    Wo = np.asarray(Wo, np.float32)
    bo = np.asarray(bo, np.float32)
    # shared across cores: per-batch transposed bf16 inputs, packed Wo
    xq_b = [Q[b].T.astype(bf) for b in range(B)]
    xk_b = [K[b].T.astype(bf) for b in range(B)]
    xv_b = [V[b].T.astype(bf) for b in range(B)]
    wo_b = [_pack_wo(Wo, b) for b in range(B)]
    bor = np.ascontiguousarray(np.broadcast_to(bo, (128, DIN)))
    # per head group g: projection weights/biases
    wq_g, wk_g, wv_g, bq_g, bk_g, bv_g = [], [], [], [], [], []
    for g in range(4):
        hs = slice(HL * g, HL * (g + 1))
        wq_g.append(Wq[hs].transpose(1, 0, 2).reshape(DIN, HL * DK).astype(bf))
        wk_g.append(Wk[hs].transpose(1, 0, 2).reshape(DIN, HL * DK).astype(bf))
        wv_g.append(Wv[hs].transpose(1, 0, 2).reshape(DIN, HL * DK).astype(bf))
        bq_g.append(np.ascontiguousarray(bq[hs].reshape(2, 128).T))
        bk_g.append(np.ascontiguousarray(bk[hs].reshape(2, 128).T))
        bv_g.append(
            np.ascontiguousarray(np.broadcast_to(bv[hs].reshape(-1), (128, HL * DK)))
        )
    in_maps = []
    for c in range(NCORES):
        b, g = divmod(c, 4)
        in_maps.append(
            {
                "xqt": xq_b[b],
                "xkt": xk_b[b],
                "xvt": xv_b[b],
                "wq": wq_g[g],
                "wk": wk_g[g],
                "wv": wv_g[g],
                "wo": wo_b[b],
                "bqp": bq_g[g],
                "bkp": bk_g[g],
                "bvr": bv_g[g],
                "bor": bor,
            }
        )
    return in_maps


def run(nc, in_maps, **kwargs):
    return bass_utils.run_bass_kernel_spmd(
        nc, in_maps, core_ids=list(range(NCORES)), **kwargs
    )


def kernel(Q, K, V, Wq, bq, Wk, bk, Wv, bv, Wo, bo):
    nc = _get_nc()
    in_maps = make_in_maps(Q, K, V, Wq, bq, Wk, bk, Wv, bv, Wo, bo)
    res = run(nc, in_maps)
    full = np.empty((B, S, DIN), np.float32)
    for c in range(NCORES):
        b, g = divmod(c, 4)
        full[b, SQ * g : SQ * (g + 1), :] = res.results[c]["out"]
    return full


# revision 25
# speedup vs baseline: 1.0953x; 1.0318x over previous
"""Multi-head attention Trainium2 kernel (8 NeuronCores).

Sharding: core c handles batch b=c//4 and head group g=c%4 (4 of 16 heads).
Fully "transposed" formulation (no on-device transposes):
  qT/kT [dq, s] via lhsT=W-pair, rhs=X^T;  v [s, dk] via lhsT=X^T-chunk, rhs=Wv
  scoresT[s_k, s_q] via lhsT=kT-chunk, rhs=qT (two 64-row head MMs run
  concurrently on the PE via row tiling); exp fused PSUM->SBUF on ScalarE
  (scale=1/sqrt(dq)), with a fraction of tiles offloaded to the Vector engine
  using a Schraudolph bit-trick fast-exp (x*A+B written as int16, bitcast to
  bf16); rowsum via a ones-column block in the attn@v matmul (free: matmul
  cost depends only on moving columns); oT[dk, s_q] is exactly the lhsT the
  output projection wants.
An AllToAll across all 8 cores reshards from (4 local heads, all s) to
(all 16 heads, s-quarter); Wo is zero-padded for the other batch's shard rows
(4-rank groups are unsupported: the mesh algorithm needs >4 ranks). The pair-0
half of the output projection is emitted before the pair-1 AllToAll completes
so the PE works during the collective. Inputs are cast to bf16 host-side to
halve HBM load traffic.
"""

import sys

if "/opt/trn_rl_repo" not in sys.path:
    sys.path.insert(0, "/opt/trn_rl_repo")

import ml_dtypes
import numpy as np

import concourse.bass as bass  # noqa: F401
import concourse.bacc as bacc
import concourse.bass_utils as bass_utils
import concourse.mybir as mybir
import concourse.tile as tile

B, S, DIN = 2, 2048, 1024
H, DK = 16, 64
NCORES = 8
HL = 4  # heads per core
SQ = S // 4  # output rows per core

F32 = mybir.dt.float32
BF16 = mybir.dt.bfloat16
I16 = mybir.dt.int16

DC = DIN // 128  # 8 din chunks
SKC = S // 128  # 16 s_k chunks
VW = 2 * DK  # 128: 64 v columns + 64 ones columns (rowsum via PE)

SCALE = float(1.0 / np.sqrt(DK))
# Schraudolph fast-exp producing bf16 bit patterns: bits = int16(x*FE_A + FE_B)
FE_A = float(128.0 / np.log(2.0)) * SCALE
FE_B = float(127.0 * 128.0 - 0.5)
# skc chunks whose exp runs on the Vector engine instead of ScalarE
DVE_SKC = (1, 5, 9, 13)


def build(dbg=False):
    nc = bacc.Bacc("TRN2", target_bir_lowering=False, debug=False, num_devices=NCORES)

    xqt = nc.dram_tensor("xqt", [DIN, S], BF16, kind="ExternalInput")
    xkt = nc.dram_tensor("xkt", [DIN, S], BF16, kind="ExternalInput")
    xvt = nc.dram_tensor("xvt", [DIN, S], BF16, kind="ExternalInput")
    wq = nc.dram_tensor("wq", [DIN, HL * DK], BF16, kind="ExternalInput")
    wk = nc.dram_tensor("wk", [DIN, HL * DK], BF16, kind="ExternalInput")
    wv = nc.dram_tensor("wv", [DIN, HL * DK], BF16, kind="ExternalInput")
    # Wo in AllToAll row order with zero rows for the other batch's shards:
    # chunk 8p+i = heads (4*(i%4)+2p, +1) of rank i if i//4==b else zeros
    wo = nc.dram_tensor("wo", [2 * H * DK, DIN], BF16, kind="ExternalInput")
    bqp = nc.dram_tensor("bqp", [128, 2], F32, kind="ExternalInput")
    bkp = nc.dram_tensor("bkp", [128, 2], F32, kind="ExternalInput")
    bvr = nc.dram_tensor("bvr", [128, HL * DK], F32, kind="ExternalInput")
    bor = nc.dram_tensor("bor", [128, DIN], F32, kind="ExternalInput")
    out = nc.dram_tensor("out", [SQ, DIN], F32, kind="ExternalOutput")
    if dbg:
        d_qt = nc.dram_tensor("d_qt", [128, S], BF16, kind="ExternalOutput")
        d_kt = nc.dram_tensor("d_kt", [128, S], BF16, kind="ExternalOutput")
        d_v = nc.dram_tensor("d_v", [128, HL * VW], BF16, kind="ExternalOutput")
        d_eta = nc.dram_tensor("d_eta", [128, 1024], BF16, kind="ExternalOutput")
        d_etd = nc.dram_tensor("d_etd", [128, 1024], BF16, kind="ExternalOutput")
        d_po = nc.dram_tensor("d_po", [128, 1024], F32, kind="ExternalOutput")
        d_ot = nc.dram_tensor("d_ot", [64, 1024], BF16, kind="ExternalOutput")
        d_ol = nc.dram_tensor("d_ol", [128, 2 * DC * SQ], BF16, kind="ExternalOutput")
        d_wo = nc.dram_tensor("d_wo", [128, DIN], BF16, kind="ExternalOutput")

    with tile.TileContext(nc) as tc:
        with (
            tc.tile_pool(name="pers", bufs=1) as pers,
            tc.tile_pool(name="work", bufs=3) as work,
            tc.tile_pool(name="wrk2", bufs=2) as wrk2,
            tc.tile_pool(name="pmm", bufs=2, space="PSUM") as pmm,
            tc.tile_pool(name="pacc", bufs=2, space="PSUM") as pacc,
            tc.tile_pool(name="dram", bufs=1, space="DRAM") as dram,
        ):
            # ---- weights/biases; q/k path on sync queue, v path on gpsimd ----
            wq_sb = pers.tile([128, DC, HL * DK], BF16)
            wk_sb = pers.tile([128, DC, HL * DK], BF16)
            wv_sb = pers.tile([128, DC, HL * DK], BF16)
            bq_sb = pers.tile([128, 2], F32)
            bk_sb = pers.tile([128, 2], F32)
            bv_sb = pers.tile([128, HL * DK], F32)
            bo_sb = pers.tile([128, DIN], F32)
            nc.sync.dma_start(wq_sb[:], wq.rearrange("(c p) d -> p c d", p=128))
            nc.sync.dma_start(wk_sb[:], wk.rearrange("(c p) d -> p c d", p=128))
            nc.sync.dma_start(bq_sb[:], bqp[:])
            nc.sync.dma_start(bk_sb[:], bkp[:])
            nc.gpsimd.dma_start(wv_sb[:], wv.rearrange("(c p) d -> p c d", p=128))
            nc.gpsimd.dma_start(bv_sb[:], bvr[:])
            nc.gpsimd.dma_start(bo_sb[:], bor[:])

            # ---- X^T streamed per 512-col s-block; "big" slots recycled for
            # wo_sb / ol_sb after the projections are done ----
            xq_sb = pers.tile([128, DC, S], BF16, tag="big", bufs=3, name="xq_sb")
            xk_sb = pers.tile([128, DC, S], BF16, tag="big", bufs=3, name="xk_sb")
            xv_sb = pers.tile([128, DC, S], BF16, tag="big", bufs=3, name="xv_sb")
            for sblk in range(4):
                ssl = slice(512 * sblk, 512 * (sblk + 1))
                for xsb, xdram in ((xq_sb, xqt), (xk_sb, xkt), (xv_sb, xvt)):
                    nc.gpsimd.dma_start(
                        xsb[:, :, ssl],
                        xdram[:, ssl].rearrange("(c p) s -> p c s", p=128),
                    )

            # ---- projection targets ----
            qt_sb = [pers.tile([128, S], BF16, name=f"qt{p}") for p in range(2)]
            kt_sb = [pers.tile([128, S], BF16, name=f"kt{p}") for p in range(2)]
            v_sb = pers.tile([128, SKC, HL * VW], BF16)

            def emit_qk(p, sblks):
                for xsb, wsb, bsb, dst in (
                    (xq_sb, wq_sb, bq_sb, qt_sb),
                    (xk_sb, wk_sb, bk_sb, kt_sb),
                ):
                    for sb in sblks:
                        ps = pmm.tile([128, 1024], F32, tag="mm", name="psqk")
                        for c in range(DC):
                            nc.tensor.matmul(
                                ps[:, 0:512],
                                wsb[:, c, 128 * p : 128 * (p + 1)],
                                xsb[:, c, 512 * sb : 512 * (sb + 1)],
                                start=(c == 0),
                                stop=(c == DC - 1),
                            )
                        nc.vector.tensor_scalar_add(
                            dst[p][:, 512 * sb : 512 * (sb + 1)],
                            ps[:, 0:512],
                            bsb[:, p : p + 1],
                        )

            def emit_v_ones():
                for hl in range(HL):
                    nc.vector.memset(v_sb[:, :, hl * VW + DK : (hl + 1) * VW], 1.0)

            def emit_v(scs):
                for sc in scs:
                    psv = pmm.tile([128, 1024], F32, tag="mm", name="psv")
                    for c in range(DC):
                        nc.tensor.matmul(
                            psv[:, 0 : HL * DK],
                            xv_sb[:, c, 128 * sc : 128 * (sc + 1)],
                            wv_sb[:, c, :],
                            start=(c == 0),
                            stop=(c == DC - 1),
                        )
                    for hl in range(HL):
                        nc.vector.tensor_add(
                            v_sb[:, sc, hl * VW : hl * VW + DK],
                            psv[:, hl * DK : (hl + 1) * DK],
                            bv_sb[:, hl * DK : (hl + 1) * DK],
                        )

            # ---- attention + collective staging ----
            # per pair: 8 shards of 128 rows (2 heads x 64); shard j carries
            # s_q quarter j%4, written twice (once per batch's rank range)
            cc_in = [dram.tile([8 * VW, SQ], BF16, name=f"cc_in{p}") for p in range(2)]
            cc_out = [
                dram.tile([8 * VW, SQ], BF16, name=f"cc_out{p}") for p in range(2)
            ]

            def emit_a2a(p):
                nc.gpsimd.collective_compute(
                    "AllToAll",
                    mybir.AluOpType.bypass,
                    replica_groups=[[0, 1, 2, 3, 4, 5, 6, 7]],
                    ins=[cc_in[p].opt()],
                    outs=[cc_out[p].opt()],
                )

            def emit_scores_exp(p, sqb, skc, eng):
                qsl = slice(512 * sqb, 512 * (sqb + 1))
                # [0:512]=head 2p scores (bank A), [512:1024]=head 2p+1
                # (bank B); shared tile -> the two 64-row MMs issue
                # adjacently and run concurrently on the PE
                ps2 = pmm.tile([128, 1024], F32, tag="mm", name="ps2")
                for ch in range(2):
                    cs = slice(64 * ch, 64 * (ch + 1))
                    nc.tensor.matmul(
                        ps2[:, 512 * ch : 512 * (ch + 1)],
                        kt_sb[p][cs, 128 * skc : 128 * (skc + 1)],
                        qt_sb[p][cs, qsl],
                        start=True,
                        stop=True,
                    )
                et = work.tile([128, 1024], BF16, tag="et", bufs=4, name="et")
                if eng == "dve":
                    nc.vector.tensor_scalar(
                        et.bitcast(I16),
                        ps2[:],
                        FE_A,
                        FE_B,
                        mybir.AluOpType.mult,
                        mybir.AluOpType.add,
                    )
                else:
                    nc.scalar.activation(
                        et[:],
                        ps2[:],
                        mybir.ActivationFunctionType.Exp,
                        bias=0.0,
                        scale=SCALE,
                    )
                if dbg and p == 0 and sqb == 1 and skc == 1:
                    nc.sync.dma_start(d_eta[:], et[:])
                if dbg and p == 0 and sqb == 1 and skc == 0:
                    nc.sync.dma_start(d_etd[:], et[:])
                return et

            def emit_av(p, skc, po, et):
                for ch in range(2):
                    hl = 2 * p + ch
                    nc.tensor.matmul(
                        po[:, 512 * ch : 512 * (ch + 1)],
                        v_sb[:, skc, hl * VW : (hl + 1) * VW],
                        et[:, 512 * ch : 512 * (ch + 1)],
                        start=(skc == 0),
                        stop=(skc == SKC - 1),
                    )

            def super_block(p, half, skcs, poA, poB, dve_every=1):
                # two s_q quarters interleaved: the PE runs chain B's scores
                # and chain A's attn@v while chain A's exp is in flight
                sqbA, sqbB = 2 * half, 2 * half + 1
                for skc in skcs:
                    etA = emit_scores_exp(p, sqbA, skc, "act")
                    engB = "dve" if skc % dve_every == 0 else "act"
                    etB = emit_scores_exp(p, sqbB, skc, engB)
                    emit_av(p, skc, poA, etA)
                    emit_av(p, skc, poB, etB)

            def emit_att_norm(p, sqb, po):
                # po rows 0:64 = oT, rows 64:128 = rowsum (ones block).
                if dbg and p == 0 and sqb == 1:
                    po_sb = wrk2.tile([128, 1024], F32, tag="rcp", name="po_sb")
                    nc.vector.tensor_copy(po_sb[:], po[:])
                    nc.sync.dma_start(d_po[:], po_sb[:])
                rcp = wrk2.tile([128, 1024], F32, tag="rcp", name="rcp")
                rlo = wrk2.tile([64, 1024], F32, tag="rlo", name="rlo")
                ot = wrk2.tile([64, 1024], BF16, tag="ot", name="ot")
                # full-tile rcp: the custom-DVE op mishandles partition-offset
                # ranges on HW (rows 0:64 are unused garbage reciprocals)
                nc.vector.reciprocal_approx_fast(out=rcp[:], in_=po[:])
                nc.sync.dma_start(rlo[:], rcp[64:128, :])
                nc.vector.tensor_mul(ot[:], po[0:DK, :], rlo[:])
                if dbg and p == 0 and sqb == 1:
                    nc.sync.dma_start(d_ot[:], ot[:])
                for shard in (sqb, sqb + 4):
                    for ch in range(2):
                        base = 128 * shard + 64 * ch
                        nc.sync.dma_start(
                            cc_in[p][base : base + 64, :],
                            ot[:, 512 * ch : 512 * (ch + 1)],
                        )

            emit_v_ones()
            # pair-0: stream projections per s-block; attention on quarters
            # (0,1) starts after two s-blocks of kt/qt have landed
            emit_qk(0, [0])
            emit_v(range(0, 4))
            emit_qk(0, [1])
            emit_v(range(4, 8))
            poA = pacc.tile([128, 1024], F32, tag="acc", name="po")
            poB = pacc.tile([128, 1024], F32, tag="acc", name="po")
            super_block(0, 0, range(0, 4), poA, poB, dve_every=2)
            emit_qk(0, [2])
            emit_v(range(8, 12))
            super_block(0, 0, range(4, 8), poA, poB, dve_every=2)
            emit_qk(0, [3])
            emit_v(range(12, 16))
            super_block(0, 0, range(8, 16), poA, poB, dve_every=2)
            emit_att_norm(0, 0, poA)
            emit_att_norm(0, 1, poB)
            emit_qk(1, [0, 1])
            poA = pacc.tile([128, 1024], F32, tag="acc", name="po")
            poB = pacc.tile([128, 1024], F32, tag="acc", name="po")
            super_block(0, 1, range(SKC), poA, poB)
            emit_att_norm(0, 2, poA)
            emit_att_norm(0, 3, poB)
            emit_qk(1, [2, 3])
            # wo load (slot freed by xq after pair-1 proj); runs during attention
            wo_sb = pers.tile([128, 2 * DC, DIN], BF16, tag="big", bufs=3, name="wo_sb")
            nc.sync.dma_start(wo_sb[:], wo.rearrange("(c p) d -> p c d", p=128))
            emit_a2a(0)  # overlaps pair-1 attention
            ol_sb = pers.tile([128, 2 * DC, SQ], BF16, tag="big", bufs=3, name="ol_sb")
            nc.gpsimd.dma_start(
                ol_sb[:, 0:DC, :], cc_out[0].rearrange("(c p) s -> p c s", p=128)
            )
            for half in range(2):
                poA = pacc.tile([128, 1024], F32, tag="acc", name="po")
                poB = pacc.tile([128, 1024], F32, tag="acc", name="po")
                super_block(1, half, range(SKC), poA, poB)
                emit_att_norm(1, 2 * half, poA)
                emit_att_norm(1, 2 * half + 1, poB)
            emit_a2a(1)

            # ---- output projection: pair-0 half runs during the pair-1
            # AllToAll; pair-1 half accumulates into the same PSUM after ----
            pso_tiles = []
            for sb2 in range(4):
                pool, tg = (pmm, "mm") if sb2 < 2 else (pacc, "acc")
                pso = pool.tile([128, 1024], F32, tag=tg, name="pso")
                for c in range(DC):
                    for do in range(2):
                        nc.tensor.matmul(
                            pso[:, 512 * do : 512 * (do + 1)],
                            ol_sb[:, c, 128 * sb2 : 128 * (sb2 + 1)],
                            wo_sb[:, c, 512 * do : 512 * (do + 1)],
                            start=(c == 0),
                            stop=False,
                        )
                pso_tiles.append(pso)
            nc.gpsimd.dma_start(
                ol_sb[:, DC : 2 * DC, :],
                cc_out[1].rearrange("(c p) s -> p c s", p=128),
            )
            if dbg:
                nc.sync.dma_start(d_qt[:], qt_sb[0][:])
                nc.sync.dma_start(d_kt[:], kt_sb[0][:])
                nc.sync.dma_start(d_v[:], v_sb[:, 0, :])
                nc.sync.dma_start(d_wo[:], wo_sb[:, 8, :])
                nc.sync.dma_start(
                    d_ol.rearrange("p (c s) -> p c s", c=2 * DC), ol_sb[:]
                )
            for sb2 in range(4):
                pso = pso_tiles[sb2]
                for c in range(DC, 2 * DC):
                    for do in range(2):
                        nc.tensor.matmul(
                            pso[:, 512 * do : 512 * (do + 1)],
                            ol_sb[:, c, 128 * sb2 : 128 * (sb2 + 1)],
                            wo_sb[:, c, 512 * do : 512 * (do + 1)],
                            start=False,
                            stop=(c == 2 * DC - 1),
                        )
                os_sb = wrk2.tile([128, DIN], F32, tag="os", name="os")
                nc.vector.tensor_add(os_sb[:], pso[:], bo_sb[:])
                nc.sync.dma_start(out[128 * sb2 : 128 * (sb2 + 1), :], os_sb[:])

    nc.compile()
    return nc


_NC = None


def _get_nc():
    global _NC
    if _NC is None:
        _NC = build()
    return _NC


def _pack_wo(Wo, b):
    """Row order matches the AllToAll output chunks: chunk 8p+i (128 rows)
    holds heads (4*(i%4)+2p, +1) of rank i. Rows for the other batch's ranks
    are zeroed (they carry that batch's data in cc_out and must not
    contribute)."""
    bf = ml_dtypes.bfloat16
    out = np.zeros((2 * H * DK, DIN), bf)
    for p in range(2):
        for i in range(8):
            if i // 4 != b:
                continue
            for hh in range(2):
                head = 4 * (i % 4) + 2 * p + hh
                dst = 1024 * p + 128 * i + 64 * hh
                out[dst : dst + 64, :] = Wo[head * 64 : (head + 1) * 64, :].astype(bf)
    return out


def make_in_maps(Q, K, V, Wq, bq, Wk, bk, Wv, bv, Wo, bo):
    bf = ml_dtypes.bfloat16
    Q, K, V = (np.asarray(a, np.float32) for a in (Q, K, V))
    Wq, bq, Wk, bk, Wv, bv = (
        np.asarray(a, np.float32) for a in (Wq, bq, Wk, bk, Wv, bv)
    )
    Wo = np.asarray(Wo, np.float32)
    bo = np.asarray(bo, np.float32)
    # shared across cores: per-batch transposed bf16 inputs, packed Wo
    xq_b = [Q[b].T.astype(bf) for b in range(B)]
    xk_b = [K[b].T.astype(bf) for b in range(B)]
    xv_b = [V[b].T.astype(bf) for b in range(B)]
    wo_b = [_pack_wo(Wo, b) for b in range(B)]
    bor = np.ascontiguousarray(np.broadcast_to(bo, (128, DIN)))
    # per head group g: projection weights/biases
    wq_g, wk_g, wv_g, bq_g, bk_g, bv_g = [], [], [], [], [], []
    for g in range(4):
        hs = slice(HL * g, HL * (g + 1))
        wq_g.append(Wq[hs].transpose(1, 0, 2).reshape(DIN, HL * DK).astype(bf))
        wk_g.append(Wk[hs].transpose(1, 0, 2).reshape(DIN, HL * DK).astype(bf))
        wv_g.append(Wv[hs].transpose(1, 0, 2).reshape(DIN, HL * DK).astype(bf))
        bq_g.append(np.ascontiguousarray(bq[hs].reshape(2, 128).T))
        bk_g.append(np.ascontiguousarray(bk[hs].reshape(2, 128).T))
        bv_g.append(
            np.ascontiguousarray(np.broadcast_to(bv[hs].reshape(-1), (128, HL * DK)))
        )
    in_maps = []
    for c in range(NCORES):
        b, g = divmod(c, 4)
        in_maps.append(
            {
                "xqt": xq_b[b],
                "xkt": xk_b[b],
                "xvt": xv_b[b],
                "wq": wq_g[g],
                "wk": wk_g[g],
                "wv": wv_g[g],
                "wo": wo_b[b],
                "bqp": bq_g[g],
                "bkp": bk_g[g],
                "bvr": bv_g[g],
                "bor": bor,
            }
        )
    return in_maps


def run(nc, in_maps, **kwargs):
    return bass_utils.run_bass_kernel_spmd(
        nc, in_maps, core_ids=list(range(NCORES)), **kwargs
    )


def kernel(Q, K, V, Wq, bq, Wk, bk, Wv, bv, Wo, bo):
    nc = _get_nc()
    in_maps = make_in_maps(Q, K, V, Wq, bq, Wk, bk, Wv, bv, Wo, bo)
    res = run(nc, in_maps)
    full = np.empty((B, S, DIN), np.float32)
    for c in range(NCORES):
        b, g = divmod(c, 4)
        full[b, SQ * g : SQ * (g + 1), :] = res.results[c]["out"]
    return full


# revision 33
# speedup vs baseline: 1.0989x; 1.0033x over previous
"""Multi-head attention Trainium2 kernel (8 NeuronCores).

Sharding: core c handles batch b=c//4 and head group g=c%4 (4 of 16 heads).
Fully "transposed" formulation (no on-device transposes):
  qT/kT [dq, s] via lhsT=W-pair, rhs=X^T;  v [s, dk] via lhsT=X^T-chunk, rhs=Wv
  scoresT[s_k, s_q] via lhsT=kT-chunk, rhs=qT (two 64-row head MMs run
  concurrently on the PE via row tiling); exp fused PSUM->SBUF on ScalarE
  (scale=1/sqrt(dq)), with a fraction of tiles offloaded to the Vector engine
  using a Schraudolph bit-trick fast-exp (x*A+B written as int16, bitcast to
  bf16); rowsum via a ones-column block in the attn@v matmul (free: matmul
  cost depends only on moving columns); oT[dk, s_q] is exactly the lhsT the
  output projection wants.
An AllToAll across all 8 cores reshards from (4 local heads, all s) to
(all 16 heads, s-quarter); Wo is zero-padded for the other batch's shard rows
(4-rank groups are unsupported: the mesh algorithm needs >4 ranks). The pair-0
half of the output projection is emitted before the pair-1 AllToAll completes
so the PE works during the collective. Inputs are cast to bf16 host-side to
halve HBM load traffic.
"""

import sys

if "/opt/trn_rl_repo" not in sys.path:
    sys.path.insert(0, "/opt/trn_rl_repo")

import ml_dtypes
import numpy as np

import concourse.bass as bass  # noqa: F401
import concourse.bacc as bacc
import concourse.bass_utils as bass_utils
import concourse.mybir as mybir
import concourse.tile as tile

B, S, DIN = 2, 2048, 1024
H, DK = 16, 64
NCORES = 8
HL = 4  # heads per core
SQ = S // 4  # output rows per core

F32 = mybir.dt.float32
BF16 = mybir.dt.bfloat16
I16 = mybir.dt.int16

DC = DIN // 128  # 8 din chunks
SKC = S // 128  # 16 s_k chunks
VW = 2 * DK  # 128: 64 v columns + 64 ones columns (rowsum via PE)

SCALE = float(1.0 / np.sqrt(DK))
# Schraudolph fast-exp producing bf16 bit patterns: bits = int16(x*FE_A + FE_B)
FE_A = float(128.0 / np.log(2.0)) * SCALE
FE_B = float(127.0 * 128.0 - 0.5)



def build(dbg=False):
    nc = bacc.Bacc("TRN2", target_bir_lowering=False, debug=False, num_devices=NCORES)

    xqt = nc.dram_tensor("xqt", [DIN, S], BF16, kind="ExternalInput")
    xkt = nc.dram_tensor("xkt", [DIN, S], BF16, kind="ExternalInput")
    xvt = nc.dram_tensor("xvt", [DIN, S], BF16, kind="ExternalInput")
    wq = nc.dram_tensor("wq", [DIN, HL * DK], BF16, kind="ExternalInput")
    wk = nc.dram_tensor("wk", [DIN, HL * DK], BF16, kind="ExternalInput")
    wv = nc.dram_tensor("wv", [DIN, HL * DK], BF16, kind="ExternalInput")
    # Wo in AllToAll row order with zero rows for the other batch's shards:
    # chunk 8p+i = heads (4*(i%4)+2p, +1) of rank i if i//4==b else zeros
    wo = nc.dram_tensor("wo", [2 * H * DK, DIN], BF16, kind="ExternalInput")
    bqp = nc.dram_tensor("bqp", [128, 2], F32, kind="ExternalInput")
    bkp = nc.dram_tensor("bkp", [128, 2], F32, kind="ExternalInput")
    bvr = nc.dram_tensor("bvr", [128, HL * DK], F32, kind="ExternalInput")
    bor = nc.dram_tensor("bor", [128, DIN], F32, kind="ExternalInput")
    out = nc.dram_tensor("out", [SQ, DIN], F32, kind="ExternalOutput")
    if dbg:
        d_qt = nc.dram_tensor("d_qt", [128, S], BF16, kind="ExternalOutput")
        d_kt = nc.dram_tensor("d_kt", [128, S], BF16, kind="ExternalOutput")
        d_v = nc.dram_tensor("d_v", [128, HL * VW], BF16, kind="ExternalOutput")
        d_eta = nc.dram_tensor("d_eta", [128, 1024], BF16, kind="ExternalOutput")
        d_etd = nc.dram_tensor("d_etd", [128, 1024], BF16, kind="ExternalOutput")
        d_po = nc.dram_tensor("d_po", [128, 1024], F32, kind="ExternalOutput")
        d_ot = nc.dram_tensor("d_ot", [64, 1024], BF16, kind="ExternalOutput")
        d_ol = nc.dram_tensor("d_ol", [128, 2 * DC * SQ], BF16, kind="ExternalOutput")
        d_wo = nc.dram_tensor("d_wo", [128, DIN], BF16, kind="ExternalOutput")

    with tile.TileContext(nc) as tc:
        with (
            tc.tile_pool(name="pers", bufs=1) as pers,
            tc.tile_pool(name="work", bufs=3) as work,
            tc.tile_pool(name="wrk2", bufs=2) as wrk2,
            tc.tile_pool(name="pmm", bufs=2, space="PSUM") as pmm,
            tc.tile_pool(name="pacc", bufs=2, space="PSUM") as pacc,
            tc.tile_pool(name="dram", bufs=1, space="DRAM") as dram,
        ):
            # ---- weights/biases; q/k path on sync queue, v path on gpsimd ----
            wq_sb = pers.tile([128, DC, HL * DK], BF16)
            wk_sb = pers.tile([128, DC, HL * DK], BF16)
            wv_sb = pers.tile([128, DC, HL * DK], BF16)
            bq_sb = pers.tile([128, 2], F32)
            bk_sb = pers.tile([128, 2], F32)
            bv_sb = pers.tile([128, HL * DK], F32)
            bo_sb = pers.tile([128, DIN], F32)
            nc.sync.dma_start(wq_sb[:], wq.rearrange("(c p) d -> p c d", p=128))
            nc.sync.dma_start(wk_sb[:], wk.rearrange("(c p) d -> p c d", p=128))
            nc.sync.dma_start(bq_sb[:], bqp[:])
            nc.sync.dma_start(bk_sb[:], bkp[:])

            # ---- X^T streamed per 512-col s-block; "big" slots recycled for
            # wo_sb / ol_sb after the projections are done.  First q/k block
            # jumps the queue so the projections can start ASAP. ----
            xq_sb = pers.tile([128, DC, S], BF16, tag="big", bufs=3, name="xq_sb")
            xk_sb = pers.tile([128, DC, S], BF16, tag="big", bufs=3, name="xk_sb")
            xv_sb = pers.tile([128, DC, S], BF16, tag="big", bufs=3, name="xv_sb")

            def emit_xblk(xsb, xdram, sblk):
                ssl = slice(512 * sblk, 512 * (sblk + 1))
                nc.gpsimd.dma_start(
                    xsb[:, :, ssl],
                    xdram[:, ssl].rearrange("(c p) s -> p c s", p=128),
                )

            emit_xblk(xq_sb, xqt, 0)
            emit_xblk(xk_sb, xkt, 0)
            nc.gpsimd.dma_start(wv_sb[:], wv.rearrange("(c p) d -> p c d", p=128))
            nc.gpsimd.dma_start(bv_sb[:], bvr[:])
            emit_xblk(xv_sb, xvt, 0)
            for sblk in range(1, 4):
                for xsb, xdram in ((xq_sb, xqt), (xk_sb, xkt), (xv_sb, xvt)):
                    emit_xblk(xsb, xdram, sblk)
            nc.gpsimd.dma_start(bo_sb[:], bor[:])

            # ---- projection targets ----
            qt_sb = [pers.tile([128, S], BF16, name=f"qt{p}") for p in range(2)]
            kt_sb = [pers.tile([128, S], BF16, name=f"kt{p}") for p in range(2)]
            v_sb = pers.tile([128, SKC, HL * VW], BF16)

            def emit_qk(p, sblks):
                for xsb, wsb, bsb, dst in (
                    (xq_sb, wq_sb, bq_sb, qt_sb),
                    (xk_sb, wk_sb, bk_sb, kt_sb),
                ):
                    for sb in sblks:
                        ps = pmm.tile([128, 1024], F32, tag="mm", name="psqk")
                        for c in range(DC):
                            nc.tensor.matmul(
                                ps[:, 0:512],
                                wsb[:, c, 128 * p : 128 * (p + 1)],
                                xsb[:, c, 512 * sb : 512 * (sb + 1)],
                                start=(c == 0),
                                stop=(c == DC - 1),
                            )
                        nc.vector.tensor_scalar_add(
                            dst[p][:, 512 * sb : 512 * (sb + 1)],
                            ps[:, 0:512],
                            bsb[:, p : p + 1],
                        )

            def emit_qk1_chain(p, sb, xsb, wsb, bsb, dst):
                ps = pmm.tile([128, 1024], F32, tag="mm", name="psqk")
                for c in range(DC):
                    nc.tensor.matmul(
                        ps[:, 0:512],
                        wsb[:, c, 128 * p : 128 * (p + 1)],
                        xsb[:, c, 512 * sb : 512 * (sb + 1)],
                        start=(c == 0),
                        stop=(c == DC - 1),
                    )
                nc.vector.tensor_scalar_add(
                    dst[p][:, 512 * sb : 512 * (sb + 1)],
                    ps[:, 0:512],
                    bsb[:, p : p + 1],
                )

            def emit_v_ones():
                for hl in range(HL):
                    nc.vector.memset(v_sb[:, :, hl * VW + DK : (hl + 1) * VW], 1.0)

            def emit_v(scs):
                for sc in scs:
                    psv = pmm.tile([128, 1024], F32, tag="mm", name="psv")
                    for c in range(DC):
                        nc.tensor.matmul(
                            psv[:, 0 : HL * DK],
                            xv_sb[:, c, 128 * sc : 128 * (sc + 1)],
                            wv_sb[:, c, :],
                            start=(c == 0),
                            stop=(c == DC - 1),
                        )
                    for hl in range(HL):
                        nc.vector.tensor_add(
                            v_sb[:, sc, hl * VW : hl * VW + DK],
                            psv[:, hl * DK : (hl + 1) * DK],
                            bv_sb[:, hl * DK : (hl + 1) * DK],
                        )

            # ---- attention + collective staging ----
            # per pair: 8 shards of 128 rows (2 heads x 64); shard j carries
            # s_q quarter j%4, written twice (once per batch's rank range)
            cc_in = [dram.tile([8 * VW, SQ], BF16, name=f"cc_in{p}") for p in range(2)]
            cc_out = [
                dram.tile([8 * VW, SQ], BF16, name=f"cc_out{p}") for p in range(2)
            ]

            def emit_a2a(p):
                nc.gpsimd.collective_compute(
                    "AllToAll",
                    mybir.AluOpType.bypass,
                    replica_groups=[[0, 1, 2, 3, 4, 5, 6, 7]],
                    ins=[cc_in[p].opt()],
                    outs=[cc_out[p].opt()],
                )

            def emit_scores_exp(p, sqb, skc, eng):
                qsl = slice(512 * sqb, 512 * (sqb + 1))
                # [0:512]=head 2p scores (bank A), [512:1024]=head 2p+1
                # (bank B); shared tile -> the two 64-row MMs issue
                # adjacently and run concurrently on the PE
                ps2 = pmm.tile([128, 1024], F32, tag="mm", name="ps2")
                for ch in range(2):
                    cs = slice(64 * ch, 64 * (ch + 1))
                    nc.tensor.matmul(
                        ps2[:, 512 * ch : 512 * (ch + 1)],
                        kt_sb[p][cs, 128 * skc : 128 * (skc + 1)],
                        qt_sb[p][cs, qsl],
                        start=True,
                        stop=True,
                    )
                et = work.tile([128, 1024], BF16, tag="et", bufs=4, name="et")
                if eng == "dve":
                    nc.vector.tensor_scalar(
                        et.bitcast(I16),
                        ps2[:],
                        FE_A,
                        FE_B,
                        mybir.AluOpType.mult,
                        mybir.AluOpType.add,
                    )
                else:
                    nc.scalar.activation(
                        et[:],
                        ps2[:],
                        mybir.ActivationFunctionType.Exp,
                        bias=0.0,
                        scale=SCALE,
                    )
                if dbg and p == 0 and sqb == 1 and skc == 1:
                    nc.sync.dma_start(d_eta[:], et[:])
                if dbg and p == 0 and sqb == 1 and skc == 0:
                    nc.sync.dma_start(d_etd[:], et[:])
                return et

            def emit_av(p, skc, po, et):
                for ch in range(2):
                    hl = 2 * p + ch
                    nc.tensor.matmul(
                        po[:, 512 * ch : 512 * (ch + 1)],
                        v_sb[:, skc, hl * VW : (hl + 1) * VW],
                        et[:, 512 * ch : 512 * (ch + 1)],
                        start=(skc == 0),
                        stop=(skc == SKC - 1),
                    )

            def super_block(p, half, skcs, poA, poB, dve_every=None, fillers=None):
                # two s_q quarters interleaved: the PE runs chain B's scores
                # and chain A's attn@v while chain A's exp is in flight.
                # fillers: skc -> [callable] emitted after that iteration
                # (lazy norms of the previous superblock, proj chains).
                sqbA, sqbB = 2 * half, 2 * half + 1
                for skc in skcs:
                    etA = emit_scores_exp(p, sqbA, skc, "act")
                    if dve_every is not None:
                        engB = "dve" if skc % dve_every == 0 else "act"
                    else:
                        engB = "act" if skc % 8 == 7 else "dve"
                    etB = emit_scores_exp(p, sqbB, skc, engB)
                    emit_av(p, skc, poA, etA)
                    emit_av(p, skc, poB, etB)
                    if fillers:
                        for fn in fillers.get(skc, ()):
                            fn()

            norm_st = {}

            def emit_ot_stores(p, sqb, ot):
                if dbg and p == 0 and sqb == 1:
                    nc.sync.dma_start(d_ot[:], ot[:])
                for shard in (sqb, sqb + 4):
                    for ch in range(2):
                        base = 128 * shard + 64 * ch
                        nc.sync.dma_start(
                            cc_in[p][base : base + 64, :],
                            ot[:, 512 * ch : 512 * (ch + 1)],
                        )

            def emit_po_copy(p, sqb, po):
                # evict po to SBUF so the PSUM slot frees immediately; the
                # normalization runs lazily off the copy inside the next
                # superblock (avoids a PE stall on the pacc slot)
                po_sb = wrk2.tile([128, 1024], F32, tag="posb", name="po_sb")
                nc.vector.tensor_copy(po_sb[:], po[:])
                norm_st[(p, sqb)] = po_sb

            def emit_norm_rcp(p, sqb):
                src = norm_st[(p, sqb)]
                if dbg and p == 0 and sqb == 1:
                    nc.sync.dma_start(d_po[:], src[:])
                # full-tile rcp: the custom-DVE op mishandles partition-offset
                # ranges on HW (rows 0:64 are unused garbage reciprocals)
                rcp = wrk2.tile([128, 1024], F32, tag="rcp", name="rcp")
                rlo = wrk2.tile([64, 1024], F32, tag="rlo", name="rlo")
                nc.vector.reciprocal_approx_fast(out=rcp[:], in_=src[:])
                nc.sync.dma_start(rlo[:], rcp[64:128, :])
                norm_st[(p, sqb, "rlo")] = rlo

            def emit_norm_mul(p, sqb):
                src = norm_st.pop((p, sqb))
                rlo = norm_st.pop((p, sqb, "rlo"))
                ot = wrk2.tile([64, 1024], BF16, tag="ot", name="ot")
                nc.vector.tensor_mul(ot[:], src[0:DK, :], rlo[:])
                emit_ot_stores(p, sqb, ot)

            def emit_att_norm(p, sqb, po):
                # direct variant (used for the final superblock, straight off
                # PSUM with no eviction copy)
                rcp = wrk2.tile([128, 1024], F32, tag="rcp", name="rcp")
                rlo = wrk2.tile([64, 1024], F32, tag="rlo", name="rlo")
                ot = wrk2.tile([64, 1024], BF16, tag="ot", name="ot")
                nc.vector.reciprocal_approx_fast(out=rcp[:], in_=po[:])
                nc.sync.dma_start(rlo[:], rcp[64:128, :])
                nc.vector.tensor_mul(ot[:], po[0:DK, :], rlo[:])
                emit_ot_stores(p, sqb, ot)

            emit_v_ones()
            # pair-0: stream projections per s-block; attention on quarters
            # (0,1) starts after two s-blocks of kt/qt have landed
            emit_qk(0, [0])
            emit_v(range(0, 4))
            emit_qk(0, [1])
            emit_v(range(4, 8))
            poA = pacc.tile([128, 1024], F32, tag="acc", name="po")
            poB = pacc.tile([128, 1024], F32, tag="acc", name="po")
            super_block(0, 0, range(0, 4), poA, poB, dve_every=2)
            emit_qk(0, [2])
            emit_v(range(8, 12))
            super_block(0, 0, range(4, 8), poA, poB, dve_every=2)
            emit_qk(0, [3])
            emit_v(range(12, 16))
            super_block(0, 0, range(8, 16), poA, poB, dve_every=2)
            emit_po_copy(0, 0, poA)
            emit_po_copy(0, 1, poB)

            def norm_fillers(p, sqbA, sqbB):
                return {
                    1: [lambda: emit_norm_rcp(p, sqbA)],
                    3: [lambda: emit_norm_mul(p, sqbA)],
                    5: [lambda: emit_norm_rcp(p, sqbB)],
                    7: [lambda: emit_norm_mul(p, sqbB)],
                }

            # pair-1 q/k proj chains stream through the second pair-0
            # superblock (one 8-matmul chain per iteration)
            fill01 = norm_fillers(0, 0, 1)
            for i, (pp, sb) in enumerate(
                [(1, 0), (1, 0), (1, 1), (1, 1), (1, 2), (1, 2), (1, 3), (1, 3)]
            ):
                which = [(xq_sb, wq_sb, bq_sb, qt_sb), (xk_sb, wk_sb, bk_sb, kt_sb)]
                xsb, wsb, bsb, dst = which[i % 2]
                fill01.setdefault(8 + i, []).append(
                    lambda xsb=xsb, wsb=wsb, bsb=bsb, dst=dst, pp=pp, sb=sb: (
                        emit_qk1_chain(pp, sb, xsb, wsb, bsb, dst)
                    )
                )
            poA = pacc.tile([128, 1024], F32, tag="acc", name="po")
            poB = pacc.tile([128, 1024], F32, tag="acc", name="po")
            super_block(0, 1, range(SKC), poA, poB, fillers=fill01)
            # direct norms: the pair-0 AllToAll is emitted next and must see
            # every cc_in[0] store before it in program order
            emit_att_norm(0, 2, poA)
            emit_att_norm(0, 3, poB)
            # wo load (slot freed by xq after pair-1 proj); runs during attention
            wo_sb = pers.tile([128, 2 * DC, DIN], BF16, tag="big", bufs=3, name="wo_sb")
            nc.sync.dma_start(wo_sb[:], wo.rearrange("(c p) d -> p c d", p=128))
            emit_a2a(0)  # overlaps pair-1 attention
            ol_sb = pers.tile([128, 2 * DC, SQ], BF16, tag="big", bufs=3, name="ol_sb")
            nc.gpsimd.dma_start(
                ol_sb[:, 0:DC, :], cc_out[0].rearrange("(c p) s -> p c s", p=128)
            )
            poA = pacc.tile([128, 1024], F32, tag="acc", name="po")
            poB = pacc.tile([128, 1024], F32, tag="acc", name="po")
            super_block(1, 0, range(SKC), poA, poB)
            emit_po_copy(1, 0, poA)
            emit_po_copy(1, 1, poB)
            poA = pacc.tile([128, 1024], F32, tag="acc", name="po")
            poB = pacc.tile([128, 1024], F32, tag="acc", name="po")
            super_block(1, 1, range(SKC), poA, poB, fillers=norm_fillers(1, 0, 1))
            emit_att_norm(1, 2, poA)
            emit_att_norm(1, 3, poB)
            emit_a2a(1)

            # ---- output projection: pair-0 half runs during the pair-1
            # AllToAll; pair-1 half accumulates into the same PSUM after ----
            pso_tiles = []
            for sb2 in range(4):
                pool, tg = (pmm, "mm") if sb2 < 2 else (pacc, "acc")
                pso = pool.tile([128, 1024], F32, tag=tg, name="pso")
                for c in range(DC):
                    for do in range(2):
                        nc.tensor.matmul(
                            pso[:, 512 * do : 512 * (do + 1)],
                            ol_sb[:, c, 128 * sb2 : 128 * (sb2 + 1)],
                            wo_sb[:, c, 512 * do : 512 * (do + 1)],
                            start=(c == 0),
                            stop=False,
                        )
                pso_tiles.append(pso)
            nc.gpsimd.dma_start(
                ol_sb[:, DC : 2 * DC, :],
                cc_out[1].rearrange("(c p) s -> p c s", p=128),
            )
            if dbg:
                nc.sync.dma_start(d_qt[:], qt_sb[0][:])
                nc.sync.dma_start(d_kt[:], kt_sb[0][:])
                nc.sync.dma_start(d_v[:], v_sb[:, 0, :])
                nc.sync.dma_start(d_wo[:], wo_sb[:, 8, :])
                nc.sync.dma_start(
                    d_ol.rearrange("p (c s) -> p c s", c=2 * DC), ol_sb[:]
                )
            for sb2 in range(4):
                pso = pso_tiles[sb2]
                for c in range(DC, 2 * DC):
                    for do in range(2):
                        nc.tensor.matmul(
                            pso[:, 512 * do : 512 * (do + 1)],
                            ol_sb[:, c, 128 * sb2 : 128 * (sb2 + 1)],
                            wo_sb[:, c, 512 * do : 512 * (do + 1)],
                            start=False,
                            stop=(c == 2 * DC - 1),
                        )
                os_sb = wrk2.tile([128, DIN], F32, tag="os", name="os")
                nc.vector.tensor_add(os_sb[:], pso[:], bo_sb[:])
                nc.sync.dma_start(out[128 * sb2 : 128 * (sb2 + 1), :], os_sb[:])

    nc.compile()
    return nc


_NC = None


def _get_nc():
    global _NC
    if _NC is None:
        _NC = build()
    return _NC


def _pack_wo(Wo, b):
    """Row order matches the AllToAll output chunks: chunk 8p+i (128 rows)
    holds heads (4*(i%4)+2p, +1) of rank i. Rows for the other batch's ranks
    are zeroed (they carry that batch's data in cc_out and must not
    contribute)."""
    bf = ml_dtypes.bfloat16
    out = np.zeros((2 * H * DK, DIN), bf)
    for p in range(2):
        for i in range(8):
            if i // 4 != b:
                continue
            for hh in range(2):
                head = 4 * (i % 4) + 2 * p + hh
                dst = 1024 * p + 128 * i + 64 * hh
                out[dst : dst + 64, :] = Wo[head * 64 : (head + 1) * 64, :].astype(bf)
    return out


def make_in_maps(Q, K, V, Wq, bq, Wk, bk, Wv, bv, Wo, bo):
    bf = ml_dtypes.bfloat16
    Q, K, V = (np.asarray(a, np.float32) for a in (Q, K, V))
    Wq, bq, Wk, bk, Wv, bv = (
        np.asarray(a, np.float32) for a in (Wq, bq, Wk, bk, Wv, bv)
    )
    Wo = np.asarray(Wo, np.float32)
    bo = np.asarray(bo, np.float32)
    # shared across cores: per-batch transposed bf16 inputs, packed Wo
    xq_b = [Q[b].T.astype(bf) for b in range(B)]
    xk_b = [K[b].T.astype(bf) for b in range(B)]
    xv_b = [V[b].T.astype(bf) for b in range(B)]
    wo_b = [_pack_wo(Wo, b) for b in range(B)]
    bor = np.ascontiguousarray(np.broadcast_to(bo, (128, DIN)))
    # per head group g: projection weights/biases
    wq_g, wk_g, wv_g, bq_g, bk_g, bv_g = [], [], [], [], [], []
    for g in range(4):
        hs = slice(HL * g, HL * (g + 1))
        wq_g.append(Wq[hs].transpose(1, 0, 2).reshape(DIN, HL * DK).astype(bf))
        wk_g.append(Wk[hs].transpose(1, 0, 2).reshape(DIN, HL * DK).astype(bf))
        wv_g.append(Wv[hs].transpose(1, 0, 2).reshape(DIN, HL * DK).astype(bf))
        bq_g.append(np.ascontiguousarray(bq[hs].reshape(2, 128).T))
        bk_g.append(np.ascontiguousarray(bk[hs].reshape(2, 128).T))
        bv_g.append(
            np.ascontiguousarray(np.broadcast_to(bv[hs].reshape(-1), (128, HL * DK)))
        )
    in_maps = []
    for c in range(NCORES):
        b, g = divmod(c, 4)
        in_maps.append(
            {
                "xqt": xq_b[b],
                "xkt": xk_b[b],
                "xvt": xv_b[b],
                "wq": wq_g[g],
                "wk": wk_g[g],
                "wv": wv_g[g],
                "wo": wo_b[b],
                "bqp": bq_g[g],
                "bkp": bk_g[g],
                "bvr": bv_g[g],
                "bor": bor,
            }
        )
    return in_maps


def run(nc, in_maps, **kwargs):
    return bass_utils.run_bass_kernel_spmd(
        nc, in_maps, core_ids=list(range(NCORES)), **kwargs
    )


def kernel(Q, K, V, Wq, bq, Wk, bk, Wv, bv, Wo, bo):
    nc = _get_nc()
    in_maps = make_in_maps(Q, K, V, Wq, bq, Wk, bk, Wv, bv, Wo, bo)
    res = run(nc, in_maps)
    full = np.empty((B, S, DIN), np.float32)
    for c in range(NCORES):
        b, g = divmod(c, 4)
        full[b, SQ * g : SQ * (g + 1), :] = res.results[c]["out"]
    return full
